# revision 15
# baseline (speedup 1.0000x reference)
"""Trainium2 Bass kernel for the controlled-U (CU) gate application.

Math: the reference builds U = P0 (x) I (x) ... + P1 (x) Mexp (x) I ...
with dim=2, wires=12, index=(0,1), control_state=(1,). This factors as

    U = diag(I_2048, Mexp (x) I_1024)        (4096 x 4096)

so U @ x is:
    out[0:2048]     = x[0:2048]                        (identity)
    out[2048:3072]  = c00 * x[2048:3072] + c01 * x[3072:4096]
    out[3072:4096]  = c10 * x[2048:3072] + c11 * x[3072:4096]

with [[c00, c01], [c10, c11]] = Mexp = expm(M - M^H), a 2x2 unitary
computed exactly on host (eigendecomposition of the 2x2 Hermitian
generator).

Device strategy (8 NeuronCores, SPMD, fp16 streaming; the rel-err
budget of 2e-2 dwarfs fp16's ~5e-4, so all payload traffic is 16-bit,
halving HBM bytes vs fp32):
  - core d owns top rows [256d, 256d+256) (identity) and the bottom
    pair rows [2048+128d, +128) / [3072+128d, +128).
  - the host packs per-core inputs into two [128, 4096] fp16 DRAM
    tensors. in_top is the identity payload; the kernel moves it to
    out_top with a single direct HBM->HBM DMA (flat 1 MiB spray, no
    SBUF round trip, no engine work, no semaphore dependencies).
    in_bot interleaves the four bottom planes (xr1, xi1, xr2, xi2) by
    32-row groups so that one 128x128 stationary matrix W (16 diagonal
    32x32 blocks holding the real 4x4 mix coefficients) turns each
    [128, 512] moving tile into all four output planes at once:
        out[32b+j, n] = sum_a G[b][a] * in[32a+j, n]
  - 8 fp16 matmuls (512 cols each, one PSUM bank each, 8 banks total),
    PSUM -> SBUF eviction with fp32->fp16 convert alternating between
    the ACT and DVE engines so both stay far below the DMA roofline.
  - bottom loads ride the sync HWDGE ring (the HBM->HBM passthrough is
    queued behind them), bottom stores the ACT HWDGE ring; payload
    transfers are >=512 KiB with >=4 KiB/partition descriptors (finer
    splits measured slower: 2-3 KiB descriptors drop DMA throughput
    from ~330 to ~280 GB/s).
  - outputs are fp16; the host upcasts and reassembles the complex64
    result (gather/unshard), which does not touch device time.
"""

import numpy as np

import concourse.bacc as bacc
import concourse.mybir as mybir
from concourse.tile import TileContext
from concourse.bass_utils import run_bass_kernel_spmd

# Problem geometry (hardcoded per the task contract).
D = 4096           # state dimension 2**12
B = 1024           # batch
NCORES = 8
P = 128            # SBUF partitions
TROWS = D // 2 // NCORES   # 256 top (identity) rows per core
PROWS = D // 4 // NCORES   # 128 bottom pair rows per core
F16 = mybir.dt.float16
F32 = mybir.dt.float32

NCOL = 4 * B       # 4096 packed columns per [128, NCOL] payload tensor
MMCOL = 512        # moving columns per matmul (= one PSUM bank of fp32)
NMM = NCOL // MMCOL


def _build_nc() -> bacc.Bacc:
    """Build the per-core Bass/Tile program (identical on all 8 cores)."""
    # Bacc (not raw Bass): its compile() lowers multi-dependency sync waits
    # through event semaphores — raw Bass trips walrus's per-instruction
    # wait-slot limit ("Too many sync wait commands").
    nc = bacc.Bacc("TRN2", enable_partition_id=False)

    in_top = nc.dram_tensor("in_top", [P, NCOL], F16, kind="ExternalInput")
    in_bot = nc.dram_tensor("in_bot", [P, NCOL], F16, kind="ExternalInput")
    wmat = nc.dram_tensor("wmat", [P, P], F16, kind="ExternalInput")

    out_top = nc.dram_tensor("out_top", [P, NCOL], F16, kind="ExternalOutput")
    out_bot = nc.dram_tensor("out_bot", [P, NCOL], F16, kind="ExternalOutput")

    with TileContext(nc) as tc:
        with (
            tc.tile_pool(name="const", bufs=1) as const_pool,
            tc.tile_pool(name="io", bufs=1) as io_pool,
            tc.tile_pool(name="psum", bufs=NMM, space="PSUM") as psum_pool,
        ):
            # stationary mix matrix on the ACT ring (empty at start) so the
            # sync ring's first payload load issues immediately.
            w_sb = const_pool.tile([P, P], F16)
            nc.scalar.dma_start(w_sb[:], wmat[:])

            t_bot = io_pool.tile([P, NCOL], F16, tag="t_bot")
            t_out = io_pool.tile([P, NCOL], F16, tag="t_out")

            # payload loads, sync ring: bottom first (it gates compute).
            # Two 512 KiB halves (4 KiB/partition descriptors): finer splits
            # measured slower (2-3 KiB descriptors drop the DMA from ~330 to
            # ~280 GB/s, outweighing the earlier gating semaphores).
            half = NCOL // 2
            for c in range(2):
                cs = slice(c * half, (c + 1) * half)
                nc.sync.dma_start(t_bot[:, cs], in_bot[:, cs])
            # identity passthrough as a single direct HBM->HBM copy queued
            # behind the loads. It has no semaphore dependencies, so its
            # packets drain the moment the loads finish — no SBUF round
            # trip, and no load-receipt -> store-issue latency chain at the
            # tail of the kernel. (The contiguous [128, 4096] AP collapses
            # to a flat 1 MiB transfer.)
            nc.sync.dma_start(out_top[:, :], in_top[:, :])

            # 8 matmuls; each fills one PSUM bank with all 4 output planes
            # for one 32-row x 512-col chunk. Eviction alternates ACT/DVE.
            for h in range(NMM):
                hs = slice(h * MMCOL, (h + 1) * MMCOL)
                ps = psum_pool.tile([P, MMCOL], F32, tag="ps")
                nc.tensor.matmul(ps[:], w_sb[:], t_bot[:, hs],
                                 start=True, stop=True)
                if h % 2 == 0:
                    nc.scalar.copy(t_out[:, hs], ps[:])
                else:
                    nc.vector.tensor_copy(t_out[:, hs], ps[:])
                if h % 4 == 3:
                    # store the finished 2048-col half on the ACT ring
                    ss = slice((h - 3) * MMCOL, (h + 1) * MMCOL)
                    nc.scalar.dma_start(out_bot[:, ss], t_out[:, ss])



    nc.finalize()
    return nc


_NC_CACHE = None


def _get_nc() -> bacc.Bacc:
    global _NC_CACHE
    if _NC_CACHE is None:
        _NC_CACHE = _build_nc()
    return _NC_CACHE


def _mix_matrix(M_re: np.ndarray, M_im: np.ndarray) -> np.ndarray:
    """Host-side 2x2 expm of the anti-Hermitian generator -> 128x128 fp16
    stationary matrix W with W[32a+j, 32b+j] = G[b][a] (matmul computes
    W.T @ moving, i.e. out[32b+j] = sum_a G[b][a] * in[32a+j])."""
    M = M_re.astype(np.float64) + 1j * M_im.astype(np.float64)
    A = M - M.conj().T          # anti-Hermitian
    H = -1j * A                 # Hermitian
    w, V = np.linalg.eigh(H)
    Mexp = V @ np.diag(np.exp(1j * w)) @ V.conj().T   # expm(A), exact
    c00, c01 = Mexp[0, 0], Mexp[0, 1]
    c10, c11 = Mexp[1, 0], Mexp[1, 1]
    G = np.array([
        [c00.real, -c00.imag, c01.real, -c01.imag],
        [c00.imag,  c00.real, c01.imag,  c01.real],
        [c10.real, -c10.imag, c11.real, -c11.imag],
        [c10.imag,  c10.real, c11.imag,  c11.real],
    ])
    W = np.zeros((P, P), dtype=np.float32)
    j = np.arange(32)
    for a in range(4):
        for b in range(4):
            W[32 * a + j, 32 * b + j] = G[b, a]
    return W.astype(np.float16)


def _build_in_maps(M_re, M_im, x_re, x_im) -> list[dict]:
    """fp16-quantize + pack the full inputs into per-core tensors."""
    W = _mix_matrix(np.asarray(M_re, np.float32), np.asarray(M_im, np.float32))
    xr = np.asarray(x_re, np.float32).astype(np.float16)
    xi = np.asarray(x_im, np.float32).astype(np.float16)

    in_maps = []
    for d in range(NCORES):
        t0 = d * TROWS
        b1 = D // 2 + d * PROWS
        b2 = 3 * D // 4 + d * PROWS
        in_top = np.empty((P, NCOL), np.float16)
        in_top[:, : NCOL // 2] = (
            xr[t0 : t0 + TROWS].reshape(2, P, B).transpose(1, 0, 2).reshape(P, 2 * B)
        )
        in_top[:, NCOL // 2 :] = (
            xi[t0 : t0 + TROWS].reshape(2, P, B).transpose(1, 0, 2).reshape(P, 2 * B)
        )
        planes = np.stack([
            xr[b1 : b1 + PROWS], xi[b1 : b1 + PROWS],
            xr[b2 : b2 + PROWS], xi[b2 : b2 + PROWS],
        ])  # [4, 128, 1024]
        in_bot = np.ascontiguousarray(
            planes.reshape(4, 4, 32, B).transpose(0, 2, 1, 3).reshape(P, NCOL)
        )
        in_maps.append({"in_top": in_top, "in_bot": in_bot, "wmat": W})
    return in_maps


def _assemble(results: list[dict]) -> np.ndarray:
    """Gather/unshard: upcast fp16 per-core outputs into the complex64
    full-shape result."""
    full = np.empty((D, B), dtype=np.complex64)
    for d, r in enumerate(results):
        t0 = d * TROWS
        b1 = D // 2 + d * PROWS
        b2 = 3 * D // 4 + d * PROWS
        ot = r["out_top"]
        full.real[t0 : t0 + TROWS] = (
            ot[:, : NCOL // 2].reshape(P, 2, B).transpose(1, 0, 2).reshape(TROWS, B)
        )
        full.imag[t0 : t0 + TROWS] = (
            ot[:, NCOL // 2 :].reshape(P, 2, B).transpose(1, 0, 2).reshape(TROWS, B)
        )
        ob = (
            r["out_bot"].reshape(4, 32, 4, B).transpose(0, 2, 1, 3).reshape(4, PROWS, B)
        )
        full.real[b1 : b1 + PROWS] = ob[0]
        full.imag[b1 : b1 + PROWS] = ob[1]
        full.real[b2 : b2 + PROWS] = ob[2]
        full.imag[b2 : b2 + PROWS] = ob[3]
    return full


def kernel(M_re, M_im, x_re, x_im) -> np.ndarray:
    in_maps = _build_in_maps(M_re, M_im, x_re, x_im)
    nc = _get_nc()
    res = run_bass_kernel_spmd(nc, in_maps, core_ids=list(range(NCORES)))
    return _assemble(res.results)  # (4096, 1024) complex64


# revision 16
# speedup vs baseline: 1.0031x; 1.0031x over previous
"""Trainium2 Bass kernel for the controlled-U (CU) gate application.

Math: the reference builds U = P0 (x) I (x) ... + P1 (x) Mexp (x) I ...
with dim=2, wires=12, index=(0,1), control_state=(1,). This factors as

    U = diag(I_2048, Mexp (x) I_1024)        (4096 x 4096)

so U @ x is:
    out[0:2048]     = x[0:2048]                        (identity)
    out[2048:3072]  = c00 * x[2048:3072] + c01 * x[3072:4096]
    out[3072:4096]  = c10 * x[2048:3072] + c11 * x[3072:4096]

with [[c00, c01], [c10, c11]] = Mexp = expm(M - M^H), a 2x2 unitary
computed exactly on host (eigendecomposition of the 2x2 Hermitian
generator).

Device strategy (8 NeuronCores, SPMD, fp16 streaming; the rel-err
budget of 2e-2 dwarfs fp16's ~5e-4, so all payload traffic is 16-bit,
halving HBM bytes vs fp32):
  - core d owns top rows [256d, 256d+256) (identity) and the bottom
    pair rows [2048+128d, +128) / [3072+128d, +128).
  - the host packs per-core inputs into two [128, 4096] fp16 DRAM
    tensors. in_top is the identity payload; the kernel moves it to
    out_top with a single direct HBM->HBM DMA (flat 1 MiB spray, no
    SBUF round trip, no engine work, no semaphore dependencies).
    in_bot interleaves the four bottom planes (xr1, xi1, xr2, xi2) by
    32-row groups so that one 128x128 stationary matrix W (16 diagonal
    32x32 blocks holding the real 4x4 mix coefficients) turns each
    [128, 512] moving tile into all four output planes at once:
        out[32b+j, n] = sum_a G[b][a] * in[32a+j, n]
  - 8 fp16 matmuls (512 cols each, one PSUM bank each, 8 banks total),
    PSUM -> SBUF eviction with fp32->fp16 convert alternating between
    the ACT and DVE engines so both stay far below the DMA roofline.
  - bottom loads ride the sync HWDGE ring (the HBM->HBM passthrough is
    queued behind them), bottom stores the ACT HWDGE ring; payload
    transfers are >=512 KiB with >=4 KiB/partition descriptors (finer
    splits measured slower: 2-3 KiB descriptors drop DMA throughput
    from ~330 to ~280 GB/s).
  - outputs are fp16; the host upcasts and reassembles the complex64
    result (gather/unshard), which does not touch device time.
"""

import numpy as np

import concourse.bacc as bacc
import concourse.mybir as mybir
from concourse.tile import TileContext
from concourse.bass_utils import run_bass_kernel_spmd

# Problem geometry (hardcoded per the task contract).
D = 4096           # state dimension 2**12
B = 1024           # batch
NCORES = 8
P = 128            # SBUF partitions
TROWS = D // 2 // NCORES   # 256 top (identity) rows per core
PROWS = D // 4 // NCORES   # 128 bottom pair rows per core
F16 = mybir.dt.float16
F32 = mybir.dt.float32

NCOL = 4 * B       # 4096 packed columns per [128, NCOL] payload tensor
MMCOL = 512        # moving columns per matmul (= one PSUM bank of fp32)
NMM = NCOL // MMCOL


def _build_nc() -> bacc.Bacc:
    """Build the per-core Bass/Tile program (identical on all 8 cores)."""
    # Bacc (not raw Bass): its compile() lowers multi-dependency sync waits
    # through event semaphores — raw Bass trips walrus's per-instruction
    # wait-slot limit ("Too many sync wait commands").
    nc = bacc.Bacc("TRN2", enable_partition_id=False)

    in_top = nc.dram_tensor("in_top", [P, NCOL], F16, kind="ExternalInput")
    in_bot = nc.dram_tensor("in_bot", [P, NCOL], F16, kind="ExternalInput")
    wmat = nc.dram_tensor("wmat", [P, P], F16, kind="ExternalInput")

    out_top = nc.dram_tensor("out_top", [P, NCOL], F16, kind="ExternalOutput")
    out_bot = nc.dram_tensor("out_bot", [P, NCOL], F16, kind="ExternalOutput")

    with TileContext(nc) as tc:
        with (
            tc.tile_pool(name="const", bufs=1) as const_pool,
            tc.tile_pool(name="io", bufs=1) as io_pool,
            tc.tile_pool(name="psum", bufs=NMM, space="PSUM") as psum_pool,
        ):
            # stationary mix matrix on the ACT ring (empty at start) so the
            # sync ring's first payload load issues immediately.
            w_sb = const_pool.tile([P, P], F16)
            nc.scalar.dma_start(w_sb[:], wmat[:])

            t_bot = io_pool.tile([P, NCOL], F16, tag="t_bot")
            t_out = io_pool.tile([P, NCOL], F16, tag="t_out")

            # payload loads, sync ring: bottom first (it gates compute).
            # Two 512 KiB halves (4 KiB/partition descriptors): finer splits
            # measured slower (2-3 KiB descriptors drop the DMA from ~330 to
            # ~280 GB/s, outweighing the earlier gating semaphores).
            half = NCOL // 2
            for c in range(2):
                cs = slice(c * half, (c + 1) * half)
                nc.sync.dma_start(t_bot[:, cs], in_bot[:, cs])
            # identity passthrough as a single direct HBM->HBM copy queued
            # behind the loads. It has no semaphore dependencies, so its
            # packets drain the moment the loads finish — no SBUF round
            # trip, and no load-receipt -> store-issue latency chain at the
            # tail of the kernel. (The contiguous [128, 4096] AP collapses
            # to a flat 1 MiB transfer.)
            nc.sync.dma_start(out_top[:, :], in_top[:, :])

            # 8 matmuls; each fills one PSUM bank with all 4 output planes
            # for one 32-row x 512-col chunk. Eviction alternates ACT/DVE.
            for h in range(NMM):
                hs = slice(h * MMCOL, (h + 1) * MMCOL)
                ps = psum_pool.tile([P, MMCOL], F32, tag="ps")
                # DoubleColumn streams two moving columns per cycle (extra
                # XBUSes), halving the 427ns-per-512-col matmul cadence that
                # is otherwise the serial chain feeding the stores.
                nc.tensor.matmul(ps[:], w_sb[:], t_bot[:, hs],
                                 start=True, stop=True,
                                 perf_mode=mybir.MatmulPerfMode.DoubleColumn)
                if h % 2 == 0:
                    nc.scalar.copy(t_out[:, hs], ps[:])
                else:
                    nc.vector.tensor_copy(t_out[:, hs], ps[:])
                if h % 4 == 3:
                    # store the finished 2048-col half on the ACT ring
                    ss = slice((h - 3) * MMCOL, (h + 1) * MMCOL)
                    nc.scalar.dma_start(out_bot[:, ss], t_out[:, ss])



    nc.finalize()
    return nc


_NC_CACHE = None


def _get_nc() -> bacc.Bacc:
    global _NC_CACHE
    if _NC_CACHE is None:
        _NC_CACHE = _build_nc()
    return _NC_CACHE


def _mix_matrix(M_re: np.ndarray, M_im: np.ndarray) -> np.ndarray:
    """Host-side 2x2 expm of the anti-Hermitian generator -> 128x128 fp16
    stationary matrix W with W[32a+j, 32b+j] = G[b][a] (matmul computes
    W.T @ moving, i.e. out[32b+j] = sum_a G[b][a] * in[32a+j])."""
    M = M_re.astype(np.float64) + 1j * M_im.astype(np.float64)
    A = M - M.conj().T          # anti-Hermitian
    H = -1j * A                 # Hermitian
    w, V = np.linalg.eigh(H)
    Mexp = V @ np.diag(np.exp(1j * w)) @ V.conj().T   # expm(A), exact
    c00, c01 = Mexp[0, 0], Mexp[0, 1]
    c10, c11 = Mexp[1, 0], Mexp[1, 1]
    G = np.array([
        [c00.real, -c00.imag, c01.real, -c01.imag],
        [c00.imag,  c00.real, c01.imag,  c01.real],
        [c10.real, -c10.imag, c11.real, -c11.imag],
        [c10.imag,  c10.real, c11.imag,  c11.real],
    ])
    W = np.zeros((P, P), dtype=np.float32)
    j = np.arange(32)
    for a in range(4):
        for b in range(4):
            W[32 * a + j, 32 * b + j] = G[b, a]
    return W.astype(np.float16)


def _build_in_maps(M_re, M_im, x_re, x_im) -> list[dict]:
    """fp16-quantize + pack the full inputs into per-core tensors."""
    W = _mix_matrix(np.asarray(M_re, np.float32), np.asarray(M_im, np.float32))
    xr = np.asarray(x_re, np.float32).astype(np.float16)
    xi = np.asarray(x_im, np.float32).astype(np.float16)

    in_maps = []
    for d in range(NCORES):
        t0 = d * TROWS
        b1 = D // 2 + d * PROWS
        b2 = 3 * D // 4 + d * PROWS
        in_top = np.empty((P, NCOL), np.float16)
        in_top[:, : NCOL // 2] = (
            xr[t0 : t0 + TROWS].reshape(2, P, B).transpose(1, 0, 2).reshape(P, 2 * B)
        )
        in_top[:, NCOL // 2 :] = (
            xi[t0 : t0 + TROWS].reshape(2, P, B).transpose(1, 0, 2).reshape(P, 2 * B)
        )
        planes = np.stack([
            xr[b1 : b1 + PROWS], xi[b1 : b1 + PROWS],
            xr[b2 : b2 + PROWS], xi[b2 : b2 + PROWS],
        ])  # [4, 128, 1024]
        in_bot = np.ascontiguousarray(
            planes.reshape(4, 4, 32, B).transpose(0, 2, 1, 3).reshape(P, NCOL)
        )
        in_maps.append({"in_top": in_top, "in_bot": in_bot, "wmat": W})
    return in_maps


def _assemble(results: list[dict]) -> np.ndarray:
    """Gather/unshard: upcast fp16 per-core outputs into the complex64
    full-shape result."""
    full = np.empty((D, B), dtype=np.complex64)
    for d, r in enumerate(results):
        t0 = d * TROWS
        b1 = D // 2 + d * PROWS
        b2 = 3 * D // 4 + d * PROWS
        ot = r["out_top"]
        full.real[t0 : t0 + TROWS] = (
            ot[:, : NCOL // 2].reshape(P, 2, B).transpose(1, 0, 2).reshape(TROWS, B)
        )
        full.imag[t0 : t0 + TROWS] = (
            ot[:, NCOL // 2 :].reshape(P, 2, B).transpose(1, 0, 2).reshape(TROWS, B)
        )
        ob = (
            r["out_bot"].reshape(4, 32, 4, B).transpose(0, 2, 1, 3).reshape(4, PROWS, B)
        )
        full.real[b1 : b1 + PROWS] = ob[0]
        full.imag[b1 : b1 + PROWS] = ob[1]
        full.real[b2 : b2 + PROWS] = ob[2]
        full.imag[b2 : b2 + PROWS] = ob[3]
    return full


def kernel(M_re, M_im, x_re, x_im) -> np.ndarray:
    in_maps = _build_in_maps(M_re, M_im, x_re, x_im)
    nc = _get_nc()
    res = run_bass_kernel_spmd(nc, in_maps, core_ids=list(range(NCORES)))
    return _assemble(res.results)  # (4096, 1024) complex64


# revision 17
# speedup vs baseline: 1.0756x; 1.0722x over previous
"""Trainium2 Bass kernel for the controlled-U (CU) gate application.

Math: the reference builds U = P0 (x) I (x) ... + P1 (x) Mexp (x) I ...
with dim=2, wires=12, index=(0,1), control_state=(1,). This factors as

    U = diag(I_2048, Mexp (x) I_1024)        (4096 x 4096)

so U @ x is:
    out[0:2048]     = x[0:2048]                        (identity)
    out[2048:3072]  = c00 * x[2048:3072] + c01 * x[3072:4096]
    out[3072:4096]  = c10 * x[2048:3072] + c11 * x[3072:4096]

with [[c00, c01], [c10, c11]] = Mexp = expm(M - M^H), a 2x2 unitary
computed exactly on host (eigendecomposition of the 2x2 Hermitian
generator).

Device strategy (8 NeuronCores, SPMD, fp16 streaming; the rel-err
budget of 2e-2 dwarfs fp16's ~5e-4, so all payload traffic is 16-bit,
halving HBM bytes vs fp32):
  - core d owns top rows [256d, 256d+256) (identity) and the bottom
    pair rows [2048+128d, +128) / [3072+128d, +128).
  - the host packs per-core inputs into two [128, 4096] fp16 DRAM
    tensors. in_top is the identity payload; the kernel moves it to
    out_top with a single direct HBM->HBM DMA (flat 1 MiB spray, no
    SBUF round trip, no engine work, no semaphore dependencies).
    in_bot interleaves the four bottom planes (xr1, xi1, xr2, xi2) by
    32-row groups so that one 128x128 stationary matrix W (16 diagonal
    32x32 blocks holding the real 4x4 mix coefficients) turns each
    [128, 512] moving tile into all four output planes at once:
        out[32b+j, n] = sum_a G[b][a] * in[32a+j, n]
  - 8 fp16 matmuls (512 cols each, one PSUM bank each, 8 banks total),
    PSUM -> SBUF eviction with fp32->fp16 convert alternating between
    the ACT and DVE engines so both stay far below the DMA roofline.
  - bottom loads ride the sync HWDGE ring (the HBM->HBM passthrough is
    queued behind them), bottom stores the ACT HWDGE ring; payload
    transfers are >=512 KiB with >=4 KiB/partition descriptors (finer
    splits measured slower: 2-3 KiB descriptors drop DMA throughput
    from ~330 to ~280 GB/s).
  - outputs are fp16; the host upcasts and reassembles the complex64
    result (gather/unshard), which does not touch device time.
"""

import numpy as np

import concourse.bacc as bacc
import concourse.mybir as mybir
from concourse.tile import TileContext
from concourse.bass_utils import run_bass_kernel_spmd

# Problem geometry (hardcoded per the task contract).
D = 4096           # state dimension 2**12
B = 1024           # batch
NCORES = 8
P = 128            # SBUF partitions
TROWS = D // 2 // NCORES   # 256 top (identity) rows per core
PROWS = D // 4 // NCORES   # 128 bottom pair rows per core
F16 = mybir.dt.float16
F32 = mybir.dt.float32

NCOL = 4 * B       # 4096 packed columns per [128, NCOL] payload tensor
MMCOL = 512        # moving columns per matmul (= one PSUM bank of fp32)
NMM = NCOL // MMCOL


def _build_nc() -> bacc.Bacc:
    """Build the per-core Bass/Tile program (identical on all 8 cores)."""
    # Bacc (not raw Bass): its compile() lowers multi-dependency sync waits
    # through event semaphores — raw Bass trips walrus's per-instruction
    # wait-slot limit ("Too many sync wait commands").
    nc = bacc.Bacc("TRN2", enable_partition_id=False)

    in_top = nc.dram_tensor("in_top", [P, NCOL], F16, kind="ExternalInput")
    in_bot = nc.dram_tensor("in_bot", [P, NCOL], F16, kind="ExternalInput")
    wmat = nc.dram_tensor("wmat", [P, P], F16, kind="ExternalInput")

    out_top = nc.dram_tensor("out_top", [P, NCOL], F16, kind="ExternalOutput")
    out_bot = nc.dram_tensor("out_bot", [P, NCOL], F16, kind="ExternalOutput")

    with TileContext(nc) as tc:
        with (
            tc.tile_pool(name="const", bufs=1) as const_pool,
            tc.tile_pool(name="io", bufs=1) as io_pool,
            tc.tile_pool(name="psum", bufs=NMM, space="PSUM") as psum_pool,
        ):
            # stationary mix matrix on the ACT ring (empty at start) so the
            # sync ring's first payload load issues immediately.
            w_sb = const_pool.tile([P, P], F16)
            nc.scalar.dma_start(w_sb[:], wmat[:])

            t_bot = io_pool.tile([P, NCOL], F16, tag="t_bot")
            t_out = io_pool.tile([P, NCOL], F16, tag="t_out")

            # payload loads, sync ring: bottom first (it gates compute).
            # Two 512 KiB halves (4 KiB/partition descriptors): finer splits
            # measured slower (2-3 KiB descriptors drop the DMA from ~330 to
            # ~280 GB/s, outweighing the earlier gating semaphores).
            half = NCOL // 2
            for c in range(2):
                cs = slice(c * half, (c + 1) * half)
                nc.sync.dma_start(t_bot[:, cs], in_bot[:, cs])
            # identity passthrough as a single direct HBM->HBM copy queued
            # behind the loads. It has no semaphore dependencies, so its
            # packets drain the moment the loads finish — no SBUF round
            # trip, and no load-receipt -> store-issue latency chain at the
            # tail of the kernel. (The contiguous [128, 4096] AP collapses
            # to a flat 1 MiB transfer.)
            nc.sync.dma_start(out_top[:, :], in_top[:, :])

            # 8 matmuls; each fills one PSUM bank with all 4 output planes
            # for one 32-row x 512-col chunk. Eviction alternates ACT/DVE.
            for h in range(NMM):
                hs = slice(h * MMCOL, (h + 1) * MMCOL)
                ps = psum_pool.tile([P, MMCOL], F32, tag="ps")
                nc.tensor.matmul(ps[:], w_sb[:], t_bot[:, hs],
                                 start=True, stop=True)
                if h % 2 == 0:
                    nc.scalar.copy(t_out[:, hs], ps[:])
                else:
                    nc.vector.tensor_copy(t_out[:, hs], ps[:])
                if h % 4 == 3:
                    # store the finished 2048-col half on the ACT ring
                    ss = slice((h - 3) * MMCOL, (h + 1) * MMCOL)
                    nc.scalar.dma_start(out_bot[:, ss], t_out[:, ss])



    nc.finalize()
    return nc


_NC_CACHE = None


def _get_nc() -> bacc.Bacc:
    global _NC_CACHE
    if _NC_CACHE is None:
        _NC_CACHE = _build_nc()
    return _NC_CACHE


def _mix_matrix(M_re: np.ndarray, M_im: np.ndarray) -> np.ndarray:
    """Host-side 2x2 expm of the anti-Hermitian generator -> 128x128 fp16
    stationary matrix W with W[32a+j, 32b+j] = G[b][a] (matmul computes
    W.T @ moving, i.e. out[32b+j] = sum_a G[b][a] * in[32a+j])."""
    M = M_re.astype(np.float64) + 1j * M_im.astype(np.float64)
    A = M - M.conj().T          # anti-Hermitian
    H = -1j * A                 # Hermitian
    w, V = np.linalg.eigh(H)
    Mexp = V @ np.diag(np.exp(1j * w)) @ V.conj().T   # expm(A), exact
    c00, c01 = Mexp[0, 0], Mexp[0, 1]
    c10, c11 = Mexp[1, 0], Mexp[1, 1]
    G = np.array([
        [c00.real, -c00.imag, c01.real, -c01.imag],
        [c00.imag,  c00.real, c01.imag,  c01.real],
        [c10.real, -c10.imag, c11.real, -c11.imag],
        [c10.imag,  c10.real, c11.imag,  c11.real],
    ])
    W = np.zeros((P, P), dtype=np.float32)
    j = np.arange(32)
    for a in range(4):
        for b in range(4):
            W[32 * a + j, 32 * b + j] = G[b, a]
    return W.astype(np.float16)


def _build_in_maps(M_re, M_im, x_re, x_im) -> list[dict]:
    """fp16-quantize + pack the full inputs into per-core tensors."""
    W = _mix_matrix(np.asarray(M_re, np.float32), np.asarray(M_im, np.float32))
    xr = np.asarray(x_re, np.float32).astype(np.float16)
    xi = np.asarray(x_im, np.float32).astype(np.float16)

    in_maps = []
    for d in range(NCORES):
        t0 = d * TROWS
        b1 = D // 2 + d * PROWS
        b2 = 3 * D // 4 + d * PROWS
        in_top = np.empty((P, NCOL), np.float16)
        in_top[:, : NCOL // 2] = (
            xr[t0 : t0 + TROWS].reshape(2, P, B).transpose(1, 0, 2).reshape(P, 2 * B)
        )
        in_top[:, NCOL // 2 :] = (
            xi[t0 : t0 + TROWS].reshape(2, P, B).transpose(1, 0, 2).reshape(P, 2 * B)
        )
        planes = np.stack([
            xr[b1 : b1 + PROWS], xi[b1 : b1 + PROWS],
            xr[b2 : b2 + PROWS], xi[b2 : b2 + PROWS],
        ])  # [4, 128, 1024]
        in_bot = np.ascontiguousarray(
            planes.reshape(4, 4, 32, B).transpose(0, 2, 1, 3).reshape(P, NCOL)
        )
        in_maps.append({"in_top": in_top, "in_bot": in_bot, "wmat": W})
    return in_maps


def _assemble(results: list[dict]) -> np.ndarray:
    """Gather/unshard: upcast fp16 per-core outputs into the complex64
    full-shape result."""
    full = np.empty((D, B), dtype=np.complex64)
    for d, r in enumerate(results):
        t0 = d * TROWS
        b1 = D // 2 + d * PROWS
        b2 = 3 * D // 4 + d * PROWS
        ot = r["out_top"]
        full.real[t0 : t0 + TROWS] = (
            ot[:, : NCOL // 2].reshape(P, 2, B).transpose(1, 0, 2).reshape(TROWS, B)
        )
        full.imag[t0 : t0 + TROWS] = (
            ot[:, NCOL // 2 :].reshape(P, 2, B).transpose(1, 0, 2).reshape(TROWS, B)
        )
        ob = (
            r["out_bot"].reshape(4, 32, 4, B).transpose(0, 2, 1, 3).reshape(4, PROWS, B)
        )
        full.real[b1 : b1 + PROWS] = ob[0]
        full.imag[b1 : b1 + PROWS] = ob[1]
        full.real[b2 : b2 + PROWS] = ob[2]
        full.imag[b2 : b2 + PROWS] = ob[3]
    return full


def kernel(M_re, M_im, x_re, x_im) -> np.ndarray:
    in_maps = _build_in_maps(M_re, M_im, x_re, x_im)
    nc = _get_nc()
    res = run_bass_kernel_spmd(nc, in_maps, core_ids=list(range(NCORES)))
    return _assemble(res.results)  # (4096, 1024) complex64
